# revision 1
# baseline (speedup 1.0000x reference)
"""Fused Conv1d(up=2) + FIR resample + bias for TRN2, data-parallel over batch.

Math (verified against the reference impulse response):
  the composite op out = FIR(conv_transpose(x, w, stride=2)) + b is a
  stride-2 polyphase filter with 5 effective taps built from w and the
  normalized FIR kernel kf = (1,3,1)/5 * 2 = (0.4, 1.2, 0.4):

    out[2i]   = x[i-1] @ A + x[i] @ B
    out[2i+1] = x[i-1] @ C + x[i] @ D + x[i+1] @ E
  with
    A = 1.2*w0 + 0.4*w1        B = 0.4*w1 + 1.2*w2
    C = 0.4*w0                 D = 0.4*w0 + 1.2*w1 + 0.4*w2
    E = 0.4*w2                 (w_s = w[s] as [inC, outC] matrices)

  Each core handles one batch element (N=8 over 8 cores). Even/odd taps are
  concatenated along the output-channel axis so each PSUM tile holds
  [128 tokens, even-256 | odd-256]; in row-major DRAM that is exactly 512
  contiguous floats per token pair, so the store DMA is fully contiguous.
"""

import numpy as np

import concourse.bass as bass
import concourse.mybir as mybir
import concourse.tile as tile
from concourse import bacc
from concourse.bass import ts
from concourse.bass_utils import run_bass_kernel_spmd

N_CORES = 8
H = 4096  # tokens per core
C = 256  # channels
P = 128  # SBUF partitions
NTILES = H // P  # 32 token tiles of 128
SUB = 8  # x is loaded in SUB sub-tiles per channel-chunk
SUBW = H // SUB  # 512 tokens per sub-tile
TILES_PER_SUB = SUBW // P  # 4

_NC_CACHE = None


def _build_nc():
    f32 = mybir.dt.float32
    f32r = mybir.dt.float32r
    nc = bacc.Bacc(
        "TRN2",
        target_bir_lowering=False,
        debug=False,
        enable_asserts=False,
        num_devices=N_CORES,
    )
    xT = nc.dram_tensor("xT", [C, H], f32r, kind="ExternalInput").ap()
    wm1 = nc.dram_tensor("wm1", [C, 2 * C], f32r, kind="ExternalInput").ap()
    w0 = nc.dram_tensor("w0", [C, 2 * C], f32r, kind="ExternalInput").ap()
    wp1 = nc.dram_tensor("wp1", [C, C], f32r, kind="ExternalInput").ap()
    bcat = nc.dram_tensor("bcat", [1, 2 * C], f32, kind="ExternalInput").ap()
    out = nc.dram_tensor("out", [H, 2 * C], f32, kind="ExternalOutput").ap()

    with tile.TileContext(nc) as tc:
        with (
            tc.tile_pool(name="consts", bufs=1) as consts,
            tc.tile_pool(name="xpool", bufs=1) as xpool,
            tc.tile_pool(name="opool", bufs=4) as opool,
            tc.tile_pool(name="psum", bufs=4, space="PSUM") as psum_pool,
        ):
            # Emission order drives Tile's scheduling priority: the first
            # matmuls need wm1 + x sub-tile 0 only, so load those first and
            # stream everything else behind them. All loads issue on the
            # Sync engine; stores issue on Scalar so a blocked store issue
            # can never head-of-line-block the x load stream.
            w_tiles = {}
            xt = {}
            # both channel-chunks of x viewed as [128, 2, H] for fused loads
            xT_v = xT.rearrange("(c p) h -> p c h", p=P)

            def load_w(name, ap, n, c):
                t = consts.tile([P, n], f32r, tag=f"{name}{c}")
                nc.sync.dma_start(t[:], ap[ts(c, P), :])
                w_tiles[(name, c)] = t

            def load_x(s):
                # one tile holds both chunks side by side in the free dim:
                # cols [0, SUBW+2) = chunk0, cols [SUBW+2, 2*(SUBW+2)) = chunk1
                t = xpool.tile([P, 2 * (SUBW + 2)], f32r, tag=f"x{s}")
                lo = s * SUBW - 1
                hi = (s + 1) * SUBW + 1
                src_lo, src_hi = max(lo, 0), min(hi, H)
                dst_lo = src_lo - lo
                tv = t[:].rearrange("p (c h) -> p c h", c=2)
                if lo < 0:
                    nc.vector.memset(tv[:, :, 0:1].bitcast(f32), 0.0)
                if hi > H:
                    nc.vector.memset(tv[:, :, SUBW + 1 : SUBW + 2].bitcast(f32), 0.0)
                nc.sync.dma_start(
                    tv[:, :, dst_lo : dst_lo + (src_hi - src_lo)],
                    xT_v[:, :, src_lo:src_hi],
                )
                xt[s] = t

            # PE warmup: junk matmuls on zeroed SBUF trip the HAM activity
            # window during the load phase, so the real matmuls run at
            # 2.4 GHz (K=8/8) from their first issue.
            junk = consts.tile([P, 2 * C], f32r, tag="junk")
            nc.vector.memset(junk[:].bitcast(f32), 0.0)
            psj = psum_pool.tile([P, 2 * C], f32, tag="psj")
            for _ in range(8):
                nc.tensor.matmul(psj[:], junk[:, :P], junk[:], start=True, stop=True)

            # starter tile replaces sub-tile 0: covers token tiles 0..3
            # (tokens [0, 514) incl. the d=+1 halo)
            STW = TILES_PER_SUB * P + 2  # 514 cols per chunk
            xstart = xpool.tile([P, 2 * STW], f32r, tag="xstart")
            xsv = xstart[:].rearrange("p (c h) -> p c h", c=2)
            nc.vector.memset(xsv[:, :, 0:1].bitcast(f32), 0.0)

            load_w("wm1", wm1, 2 * C, 0)
            nc.sync.dma_start(xsv[:, :, 1:STW], xT_v[:, :, 0 : STW - 1])
            load_w("wm1", wm1, 2 * C, 1)
            load_w("wp1", wp1, C, 0)
            load_w("wp1", wp1, C, 1)
            load_w("w0", w0, 2 * C, 0)
            load_w("w0", w0, 2 * C, 1)
            load_x(1)
            bias = consts.tile([P, 2 * C], f32, tag="bias")
            nc.sync.dma_start(bias[:], bcat.to_broadcast((P, 2 * C)))
            for s in range(2, SUB):
                load_x(s)

            # taps ordered so the last matmul into each PSUM column range
            # carries stop=True: d=-1 (full), d=+1 (odd half), d=0 (full)
            taps = (
                (-1, "wm1", 0, 2 * C),
                (1, "wp1", C, 2 * C),
                (0, "w0", 0, 2 * C),
            )
            # store batches: 4 token tiles per DMA, except the tail which is
            # split 2+2 so less data is in flight after the final matmul
            batches = [(i0, 4) for i0 in range(0, NTILES - 4, 4)] + [
                (NTILES - 4, 2),
                (NTILES - 2, 2),
            ]
            # out viewed as [128, NTILES, 512]: partition p of token tile i
            # holds out rows i*128+p (= 512 contiguous floats each)
            out_v = out.rearrange("(a p) j -> p a j", p=P)
            for i0, blen in batches:
                ot = opool.tile([P, blen * 2 * C], f32, tag="ot")
                for bi in range(blen):
                    i = i0 + bi
                    ps = psum_pool.tile([P, 2 * C], f32, tag="ps")
                    for mi, (d, wname, n0, n1) in enumerate(taps):
                        for c in range(2):
                            if i < TILES_PER_SUB:
                                off = c * STW
                                lhsT = xstart[:, off + i * P + 1 + d : off + i * P + 1 + d + P]
                            else:
                                s = i // TILES_PER_SUB
                                base = (i % TILES_PER_SUB) * P + 1
                                off = c * (SUBW + 2)
                                lhsT = xt[s][:, off + base + d : off + base + d + P]
                            rhs = w_tiles[(wname, c)][:]
                            nc.tensor.matmul(
                                ps[:, n0:n1],
                                lhsT,
                                rhs,
                                start=(mi == 0 and c == 0),
                                stop=(mi == 2 and c == 1),
                            )
                    nc.vector.tensor_add(ot[:, ts(bi, 2 * C)], ps[:], bias[:])
                nc.scalar.dma_start(
                    out_v[:, i0 : i0 + blen, :],
                    ot[:].rearrange("p (a j) -> p a j", a=blen),
                )

    nc.compile()
    return nc


def _get_nc():
    global _NC_CACHE
    if _NC_CACHE is None:
        _NC_CACHE = _build_nc()
    return _NC_CACHE


def _prep_in_maps(x, w, b):
    x = np.ascontiguousarray(np.asarray(x, np.float32))  # [8, 4096, 256]
    w = np.asarray(w, np.float32)  # [3, 256, 256] = [K, inC, outC]
    b = np.asarray(b, np.float32)  # [256]

    kf = np.asarray([1.0, 3.0, 1.0], np.float32)
    kf = kf / kf.sum() * 2.0  # (0.4, 1.2, 0.4)
    w0_, w1_, w2_ = w[0], w[1], w[2]
    A = kf[1] * w0_ + kf[0] * w1_
    B = kf[0] * w1_ + kf[1] * w2_
    Cm = kf[0] * w0_
    D = kf[0] * w0_ + kf[1] * w1_ + kf[0] * w2_
    E = kf[0] * w2_

    wm1 = np.ascontiguousarray(np.concatenate([A, Cm], axis=1))  # [256, 512]
    w0c = np.ascontiguousarray(np.concatenate([B, D], axis=1))  # [256, 512]
    wp1 = np.ascontiguousarray(E)  # [256, 256]
    bcat = np.ascontiguousarray(np.concatenate([b, b])[None, :])  # [1, 512]

    return [
        {
            "xT": np.ascontiguousarray(x[i].T),
            "wm1": wm1,
            "w0": w0c,
            "wp1": wp1,
            "bcat": bcat,
        }
        for i in range(N_CORES)
    ]


def kernel(x, w, b):
    nc = _get_nc()
    in_maps = _prep_in_maps(x, w, b)
    res = run_bass_kernel_spmd(nc, in_maps, list(range(N_CORES)))
    out = np.stack(
        [res.results[i]["out"].reshape(2 * H, C) for i in range(N_CORES)]
    )
    return out

